# revision 16
# baseline (speedup 1.0000x reference)
"""KGNN head kernel for Trainium2 (Bass/Tile), 8-core data-parallel SPMD.

Computation (per batch b):
    score[g, n] = sum_d drug[b, g, d] * rel[b, 8g+n, d]         (n in 0..8)
    out[b, g, d] = sum_n score[g, n] * ent[b, 8g+n, d]

v5 design (memory regime: HBM streaming is the roofline, ~211 us/core):
  - rel/ent/drug are DMA'd with an f32->bf16 cast on the SWDGE (gpsimd)
    path: HBM reads unchanged (the bound), SBUF writes halved. 6-deep
    prefetch decouples the input stream from compute jitter.
  - score is computed by a runtime-registered custom DVE op
    MUL_SCAN_KGNN: out = running-sum of (Src0*Src1) in fp32 -- one 1x
    DVE pass replaces separate product + reduce passes. Per-group sums
    are recovered as boundary differences of the running sum.
  - the drug operand is pre-broadcast over the neighbor axis once per
    superblock on the ACT engine (rank<=3 operand constraint of the
    custom-DVE path, step-1 operands).
  - w = ent*score split by neighbor range between GpSimd (n 0..4) and
    DVE (n 5..7); PE sums neighbors via PSUM-accumulating identity
    matmuls with strided rhs over the natural (gg n d) w layout.
  - partition p = (q t): 16 batches x 8 group-blocks per superblock;
    each partition's rel/ent slice is 16 KiB of contiguous HBM.
  - compute in 2 chunks/superblock, tapered to 4 chunks of 2 groups +
    split output DMA on the last superblock (short serial tail).
"""

import numpy as np

import concourse.bass as bass  # noqa: F401  (engine namespaces via nc)
import concourse.mybir as mybir
import concourse.tile as tile
from concourse import bacc
from concourse.bass_utils import run_bass_kernel_spmd
from concourse.masks import make_identity

F32 = mybir.dt.float32
BF16 = mybir.dt.bfloat16

N_CORES = 8
B_FULL = 2048
B_LOCAL = B_FULL // N_CORES  # 256
G = 64          # groups per sample
NN = 8          # neighbors per group
D = 64          # feature dim
S = G * NN      # 512 neighbor slots

Q = 16          # batches per superblock (partition-major)
T = 8           # group-blocks per batch (partition-minor); T*GG = G
GG = 8          # groups per group-block

LOOKAHEAD = 5   # input-DMA prefetch depth (bufs = LOOKAHEAD + 1)
N_GP = 5        # neighbors of the w-pass handled by GpSimd (rest on DVE)


def _register_mul_scan():
    """Register the fused multiply+running-sum custom DVE op (idempotent)."""
    from concourse import dve_ops
    from concourse.dve_spec import AluOp, Spec, Src0, Src1, lower, scan
    from concourse.dve_spec import _has_src1
    from concourse.dve_uop import DveOpSpec

    name = "MUL_SCAN_KGNN"
    if name in dve_ops._SUB_OPCODE_FOR_NAME:
        return next(op for op in dve_ops.OPS if op.name == name)

    def _ref(in0, in1, s0, s1, imm2):
        return np.cumsum(
            in0.astype(np.float32) * in1.astype(np.float32), axis=-1
        ).astype(np.float32)

    spec = Spec(body=scan(AluOp.ADD, Src0 * Src1), reference=_ref)
    row = dve_ops._CUSTOM_DVE_ROW_BASE + len(dve_ops.OPS)
    assert row < 0x20, "custom-DVE opcode rows exhausted"
    shas = {}
    for ver in ("v3", "v4"):
        s = DveOpSpec(
            name=name, opcode=row, uops=lower(spec, ver=ver), rd1_en=_has_src1(spec)
        )
        shas[ver] = s.sha(ver)
    op = dve_ops.DveOp(name, spec, subdim=False, uops_sha=shas)
    dve_ops.OPS.append(op)
    dve_ops._SUB_OPCODE_FOR_NAME[name] = row
    dve_ops.CUSTOM_DVE_SPECS[name] = spec
    return op


MUL_SCAN = _register_mul_scan()


def _build_nc(b_local: int = B_LOCAL) -> "bacc.Bacc":
    n_sblk = b_local // Q
    assert n_sblk * Q == b_local

    nc = bacc.Bacc("TRN2", target_bir_lowering=False, debug=False)

    drug_d = nc.dram_tensor("drug", [b_local, G, D], F32, kind="ExternalInput")
    rel_d = nc.dram_tensor("rel", [b_local, S, D], F32, kind="ExternalInput")
    ent_d = nc.dram_tensor("ent", [b_local, S, D], F32, kind="ExternalInput")
    out_d = nc.dram_tensor("out", [b_local, G, D], F32, kind="ExternalOutput")

    # partition p = (q t); rel/ent free = (gg n d) -> 16KB contiguous HBM runs
    rel_v = rel_d[:].rearrange(
        "(s q) (t gg n) d -> s (q t) (gg n d)", q=Q, t=T, gg=GG, n=NN
    )
    ent_v = ent_d[:].rearrange(
        "(s q) (t gg n) d -> s (q t) (gg n d)", q=Q, t=T, gg=GG, n=NN
    )
    drug_av = drug_d[:].rearrange(
        "(s q) (t gg) d -> (q t) s (gg d)", q=Q, t=T, gg=GG
    )
    out_v = out_d[:].rearrange("(s q) (t gg) d -> s (q t) (gg d)", q=Q, t=T, gg=GG)

    with tile.TileContext(nc) as tc:
        with (
            tc.tile_pool(name="const", bufs=1) as const_pool,
            tc.tile_pool(name="druga", bufs=1) as druga_pool,
            tc.tile_pool(name="rel", bufs=LOOKAHEAD + 1) as rel_pool,
            tc.tile_pool(name="ent", bufs=LOOKAHEAD + 1) as ent_pool,
            tc.tile_pool(name="db", bufs=4) as db_pool,
            tc.tile_pool(name="scan", bufs=3) as scan_pool,
            tc.tile_pool(name="score", bufs=4) as score_pool,
            tc.tile_pool(name="w", bufs=3) as w_pool,
            tc.tile_pool(name="outs", bufs=2) as out_pool,
            tc.tile_pool(name="psum", bufs=4, space="PSUM") as psum_pool,
        ):
            ident = const_pool.tile([128, 128], BF16)
            make_identity(nc, ident[:])

            # whole drug tensor, cast to bf16, in one DMA (16 KiB/partition)
            drug_all = druga_pool.tile([128, n_sblk * GG * D], BF16)
            nc.gpsimd.dma_start(out=drug_all[:], in_=drug_av)

            tiles = {}

            def issue_loads(sb):
                rel_t = rel_pool.tile([128, GG * NN * D], BF16)
                nc.gpsimd.dma_start(out=rel_t[:], in_=rel_v[sb])
                ent_t = ent_pool.tile([128, GG * NN * D], BF16)
                nc.gpsimd.dma_start(out=ent_t[:], in_=ent_v[sb])
                tiles[sb] = (rel_t, ent_t, None)

            def broadcast_drug(sb):
                """Materialize drug broadcast over the neighbor axis (ACT)."""
                db_t = db_pool.tile([128, GG * NN * D], BF16)
                nc.scalar.copy(
                    out=db_t[:].rearrange("p (gg n d) -> p gg n d", gg=GG, n=NN),
                    in_=drug_all[:, sb * GG * D : (sb + 1) * GG * D]
                    .rearrange("p (gg n d) -> p gg n d", gg=GG, n=1)
                    .to_broadcast([128, GG, NN, D]),
                )
                tiles[sb] = (tiles[sb][0], tiles[sb][1], db_t)

            def compute_chunk(sb, out_t, g0, ng):
                """score + weighted-sum for groups [g0, g0+ng) of superblock sb."""
                rel_t, ent_t, db_t = tiles[sb]
                ro = g0 * NN * D
                sz = ng * NN * D
                npages = ng * NN

                # fused multiply + running sum (fp32 state), one DVE pass
                scan_t = scan_pool.tile([128, sz], F32)
                nc.vector._custom_dve(
                    MUL_SCAN,
                    out=scan_t[:],
                    in0=rel_t[:, ro : ro + sz],
                    in1=db_t[:, ro : ro + sz],
                )
                r3 = scan_t[:].rearrange("p (gn d) -> p gn d", d=D)

                # segment sums = boundary differences of the running sum
                score_t = score_pool.tile([128, npages], F32)
                nc.scalar.copy(
                    out=score_t[:, 0:1].rearrange("p (x o) -> p x o", o=1),
                    in_=r3[:, 0:1, D - 1 : D],
                )
                nc.vector.tensor_tensor(
                    out=score_t[:, 1:npages].rearrange("p (x o) -> p x o", o=1),
                    in0=r3[:, 1:npages, D - 1 : D],
                    in1=r3[:, 0 : npages - 1, D - 1 : D],
                    op=mybir.AluOpType.subtract,
                )

                # w = ent * score, split across GpSimd / DVE by neighbor range
                w_t = w_pool.tile([128, sz], BF16)
                w4 = w_t[:].rearrange("p (gg n d) -> p gg n d", gg=ng, n=NN)
                ent4 = ent_t[:, ro : ro + sz].rearrange(
                    "p (gg n d) -> p gg n d", gg=ng, n=NN
                )
                sc4 = score_t[:].rearrange("p (gg n o) -> p gg n o", gg=ng, o=1)
                for eng, n0, n1 in (
                    (nc.gpsimd, 0, N_GP),
                    (nc.vector, N_GP, NN),
                ):
                    if n0 == n1:
                        continue
                    eng.tensor_tensor(
                        out=w4[:, :, n0:n1],
                        in0=ent4[:, :, n0:n1],
                        in1=sc4[:, :, n0:n1].to_broadcast([128, ng, n1 - n0, D]),
                        op=mybir.AluOpType.mult,
                    )

                psum_t = psum_pool.tile([128, ng * D], F32)
                w_nmaj = w_t[:].rearrange("p (gg n d) -> p n gg d", gg=ng, n=NN)
                for c in range(NN):
                    nc.tensor.matmul(
                        out=psum_t[:],
                        lhsT=ident[:],
                        rhs=w_nmaj[:, c : c + 1],
                        start=(c == 0),
                        stop=(c == NN - 1),
                    )
                do = g0 * D
                nc.scalar.copy(out=out_t[:, do : do + ng * D], in_=psum_t[:])

            def compute_sblk(sb):
                out_t = out_pool.tile([128, GG * D], F32)
                if sb == n_sblk - 1:
                    # tapered tail: 4 chunks of 2 groups, output in 2 halves
                    for h in range(4):
                        compute_chunk(sb, out_t, h * 2, 2)
                        if h == 1:
                            nc.sync.dma_start(
                                out=out_v[sb][:, : GG * D // 2],
                                in_=out_t[:, : GG * D // 2],
                            )
                    nc.sync.dma_start(
                        out=out_v[sb][:, GG * D // 2 :],
                        in_=out_t[:, GG * D // 2 :],
                    )
                else:
                    compute_chunk(sb, out_t, 0, 4)
                    compute_chunk(sb, out_t, 4, 4)
                    nc.sync.dma_start(out=out_v[sb], in_=out_t[:])

            for i in range(n_sblk + LOOKAHEAD):
                if i < n_sblk:
                    issue_loads(i)
                if 1 <= i < n_sblk + 1:
                    broadcast_drug(i - 1)
                if i >= LOOKAHEAD:
                    compute_sblk(i - LOOKAHEAD)

    nc.compile()
    return nc


_NC_CACHE: dict = {}


def _get_nc(b_local: int = B_LOCAL):
    if b_local not in _NC_CACHE:
        _NC_CACHE[b_local] = _build_nc(b_local)
    return _NC_CACHE[b_local]


def run_sharded(drug, rel, ent, trace: bool = False):
    """Shard batch dim across the 8 cores, run, gather. Returns
    (full output [B, G, D], BassKernelResults)."""
    drug = np.ascontiguousarray(np.asarray(drug, dtype=np.float32))
    rel = np.ascontiguousarray(np.asarray(rel, dtype=np.float32))
    ent = np.ascontiguousarray(np.asarray(ent, dtype=np.float32))
    b = drug.shape[0]
    nb = b // N_CORES
    assert nb * N_CORES == b
    nc = _get_nc(nb)
    in_maps = [
        {
            "drug": np.ascontiguousarray(drug[i * nb : (i + 1) * nb]),
            "rel": np.ascontiguousarray(rel[i * nb : (i + 1) * nb]),
            "ent": np.ascontiguousarray(ent[i * nb : (i + 1) * nb]),
        }
        for i in range(N_CORES)
    ]
    last_exc = None
    for attempt in range(3):
        try:
            res = run_bass_kernel_spmd(nc, in_maps, list(range(N_CORES)), trace=trace)
            break
        except Exception as exc:  # transient device-unrecoverable states
            last_exc = exc
            import time

            time.sleep(10 * (attempt + 1))
    else:
        raise last_exc
    out = np.concatenate([res.results[i]["out"] for i in range(N_CORES)], axis=0)
    return out, res


def kernel(drug, rel, ent):
    out, _ = run_sharded(drug, rel, ent, trace=False)
    return out


# revision 31
# speedup vs baseline: 1.0672x; 1.0672x over previous
"""KGNN head kernel for Trainium2 (Bass/Tile), 8-core data-parallel SPMD.

Computation (per batch b):
    score[g, n] = sum_d drug[b, g, d] * rel[b, 8g+n, d]         (n in 0..8)
    out[b, g, d] = sum_n score[g, n] * ent[b, 8g+n, d]

v5 design (memory regime: HBM streaming is the roofline, ~211 us/core):
  - rel/ent/drug are DMA'd with an f32->bf16 cast on the SWDGE (gpsimd)
    path: HBM reads unchanged (the bound), SBUF writes halved. 6-deep
    prefetch decouples the input stream from compute jitter.
  - score is computed by a runtime-registered custom DVE op
    MUL_SCAN_KGNN: out = running-sum of (Src0*Src1) in fp32 -- one 1x
    DVE pass replaces separate product + reduce passes. Per-group sums
    are recovered as boundary differences of the running sum.
  - the drug operand is pre-broadcast over the neighbor axis once per
    superblock on the ACT engine (rank<=3 operand constraint of the
    custom-DVE path, step-1 operands).
  - w = ent*score split by neighbor range between GpSimd (n 0..4) and
    DVE (n 5..7); PE sums neighbors via PSUM-accumulating identity
    matmuls with strided rhs over the natural (gg n d) w layout.
  - partition p = (q t): 16 batches x 8 group-blocks per superblock;
    each partition's rel/ent slice is 16 KiB of contiguous HBM.
  - compute in 2 chunks/superblock, tapered to 4 chunks of 2 groups +
    split output DMA on the last superblock (short serial tail).
"""

import numpy as np

import concourse.bass as bass  # noqa: F401  (engine namespaces via nc)
import concourse.mybir as mybir
import concourse.tile as tile
from concourse import bacc
from concourse.bass_utils import run_bass_kernel_spmd
from concourse.masks import make_identity

F32 = mybir.dt.float32
BF16 = mybir.dt.bfloat16

N_CORES = 8
B_FULL = 2048
B_LOCAL = B_FULL // N_CORES  # 256
G = 64          # groups per sample
NN = 8          # neighbors per group
D = 64          # feature dim
S = G * NN      # 512 neighbor slots

Q = 16          # batches per superblock (partition-major)
T = 8           # group-blocks per batch (partition-minor); T*GG = G
GG = 8          # groups per group-block

LOOKAHEAD = 7   # input-DMA prefetch depth (bufs = LOOKAHEAD + 1)
N_GP = 5        # neighbors of the w-pass handled by GpSimd (rest on DVE)
IDENT_DT = mybir.dt.float8e4  # identity lhsT dtype (fp8: lower PE energy)
SCAN0 = True    # write scan out through a stride-0 page dim (boundary only)
DB_BUFS = 4     # drug-broadcast pool depth
W_BY_GROUP = 0  # if >0: split w by groups (DVE takes W_BY_GROUP groups/chunk)


def _register_mul_scan():
    """Register the fused multiply+running-sum custom DVE op (idempotent)."""
    from concourse import dve_ops
    from concourse.dve_spec import AluOp, Spec, Src0, Src1, lower, scan
    from concourse.dve_spec import _has_src1
    from concourse.dve_uop import DveOpSpec

    name = "MUL_SCAN_KGNN"
    if name in dve_ops._SUB_OPCODE_FOR_NAME:
        return next(op for op in dve_ops.OPS if op.name == name)

    def _ref(in0, in1, s0, s1, imm2):
        return np.cumsum(
            in0.astype(np.float32) * in1.astype(np.float32), axis=-1
        ).astype(np.float32)

    spec = Spec(body=scan(AluOp.ADD, Src0 * Src1), reference=_ref)
    row = dve_ops._CUSTOM_DVE_ROW_BASE + len(dve_ops.OPS)
    assert row < 0x20, "custom-DVE opcode rows exhausted"
    shas = {}
    for ver in ("v3", "v4"):
        s = DveOpSpec(
            name=name, opcode=row, uops=lower(spec, ver=ver), rd1_en=_has_src1(spec)
        )
        shas[ver] = s.sha(ver)
    op = dve_ops.DveOp(name, spec, subdim=False, uops_sha=shas)
    dve_ops.OPS.append(op)
    dve_ops._SUB_OPCODE_FOR_NAME[name] = row
    dve_ops.CUSTOM_DVE_SPECS[name] = spec
    return op


MUL_SCAN = _register_mul_scan()


def _build_nc(b_local: int = B_LOCAL) -> "bacc.Bacc":
    n_sblk = b_local // Q
    assert n_sblk * Q == b_local

    nc = bacc.Bacc("TRN2", target_bir_lowering=False, debug=False)

    drug_d = nc.dram_tensor("drug", [b_local, G, D], F32, kind="ExternalInput")
    rel_d = nc.dram_tensor("rel", [b_local, S, D], F32, kind="ExternalInput")
    ent_d = nc.dram_tensor("ent", [b_local, S, D], F32, kind="ExternalInput")
    out_d = nc.dram_tensor("out", [b_local, G, D], F32, kind="ExternalOutput")

    # partition p = (q t); rel/ent free = (gg n d) -> 16KB contiguous HBM runs
    rel_v = rel_d[:].rearrange(
        "(s q) (t gg n) d -> s (q t) (gg n d)", q=Q, t=T, gg=GG, n=NN
    )
    ent_v = ent_d[:].rearrange(
        "(s q) (t gg n) d -> s (q t) (gg n d)", q=Q, t=T, gg=GG, n=NN
    )
    drug_av = drug_d[:].rearrange(
        "(s q) (t gg) d -> (q t) s (gg d)", q=Q, t=T, gg=GG
    )
    out_v = out_d[:].rearrange("(s q) (t gg) d -> s (q t) (gg d)", q=Q, t=T, gg=GG)

    with tile.TileContext(nc) as tc:
        with (
            tc.tile_pool(name="const", bufs=1) as const_pool,
            tc.tile_pool(name="druga", bufs=1) as druga_pool,
            tc.tile_pool(name="rel", bufs=LOOKAHEAD + 1) as rel_pool,
            tc.tile_pool(name="ent", bufs=LOOKAHEAD + 1) as ent_pool,
            tc.tile_pool(name="db", bufs=DB_BUFS) as db_pool,
            tc.tile_pool(name="scan", bufs=1 if SCAN0 else 3) as scan_pool,
            tc.tile_pool(name="score", bufs=6) as score_pool,
            tc.tile_pool(name="w", bufs=3) as w_pool,
            tc.tile_pool(name="outs", bufs=2) as out_pool,
            tc.tile_pool(name="psum", bufs=4, space="PSUM") as psum_pool,
        ):
            ident = const_pool.tile([128, 128], IDENT_DT or BF16)
            make_identity(nc, ident[:])

            # whole drug tensor, cast to bf16, in one DMA (16 KiB/partition);
            # issued after the first rel/ent loads so the stream starts first
            drug_all = druga_pool.tile([128, n_sblk * GG * D], BF16)

            tiles = {}

            def issue_loads(sb):
                rel_t = rel_pool.tile([128, GG * NN * D], BF16)
                nc.gpsimd.dma_start(out=rel_t[:], in_=rel_v[sb])
                ent_t = ent_pool.tile([128, GG * NN * D], BF16)
                nc.gpsimd.dma_start(out=ent_t[:], in_=ent_v[sb])
                tiles[sb] = (rel_t, ent_t, None)

            def broadcast_drug(sb):
                """Materialize drug broadcast over the neighbor axis (ACT)."""
                db_t = db_pool.tile([128, GG * NN * D], BF16)
                nc.scalar.copy(
                    out=db_t[:].rearrange("p (gg n d) -> p gg n d", gg=GG, n=NN),
                    in_=drug_all[:, sb * GG * D : (sb + 1) * GG * D]
                    .rearrange("p (gg n d) -> p gg n d", gg=GG, n=1)
                    .to_broadcast([128, GG, NN, D]),
                )
                tiles[sb] = (tiles[sb][0], tiles[sb][1], db_t)

            def compute_chunk(sb, out_t, g0, ng):
                """score + weighted-sum for groups [g0, g0+ng) of superblock sb."""
                rel_t, ent_t, db_t = tiles[sb]
                ro = g0 * NN * D
                sz = ng * NN * D
                npages = ng * NN

                # fused multiply + running sum (fp32 state), one DVE pass
                if SCAN0:
                    # stride-0 page dim: every element of a page writes the
                    # same slot; the last (the page boundary value) persists
                    e_t = score_pool.tile([128, npages], F32)
                    nc.vector._custom_dve(
                        MUL_SCAN,
                        out=e_t[:]
                        .rearrange("p (x o) -> p x o", o=1)
                        .to_broadcast([128, npages, D]),
                        in0=rel_t[:, ro : ro + sz],
                        in1=db_t[:, ro : ro + sz],
                    )
                    score_t = score_pool.tile([128, npages], F32)
                    nc.scalar.copy(
                        out=score_t[:, 0:1].rearrange("p (x o) -> p x o", o=1),
                        in_=e_t[:, 0:1].rearrange("p (x o) -> p x o", o=1),
                    )
                    nc.vector.tensor_tensor(
                        out=score_t[:, 1:npages].rearrange("p (x o) -> p x o", o=1),
                        in0=e_t[:, 1:npages].rearrange("p (x o) -> p x o", o=1),
                        in1=e_t[:, 0 : npages - 1].rearrange("p (x o) -> p x o", o=1),
                        op=mybir.AluOpType.subtract,
                    )
                else:
                    scan_t = scan_pool.tile([128, sz], F32)
                    nc.vector._custom_dve(
                        MUL_SCAN,
                        out=scan_t[:],
                        in0=rel_t[:, ro : ro + sz],
                        in1=db_t[:, ro : ro + sz],
                    )
                    r3 = scan_t[:].rearrange("p (gn d) -> p gn d", d=D)

                    # segment sums = boundary differences of the running sum
                    score_t = score_pool.tile([128, npages], F32)
                    nc.scalar.copy(
                        out=score_t[:, 0:1].rearrange("p (x o) -> p x o", o=1),
                        in_=r3[:, 0:1, D - 1 : D],
                    )
                    nc.vector.tensor_tensor(
                        out=score_t[:, 1:npages].rearrange("p (x o) -> p x o", o=1),
                        in0=r3[:, 1:npages, D - 1 : D],
                        in1=r3[:, 0 : npages - 1, D - 1 : D],
                        op=mybir.AluOpType.subtract,
                    )

                # w = ent * score, split across GpSimd / DVE
                w_t = w_pool.tile([128, sz], BF16)
                w4 = w_t[:].rearrange("p (gg n d) -> p gg n d", gg=ng, n=NN)
                ent4 = ent_t[:, ro : ro + sz].rearrange(
                    "p (gg n d) -> p gg n d", gg=ng, n=NN
                )
                sc4 = score_t[:].rearrange("p (gg n o) -> p gg n o", gg=ng, o=1)
                if W_BY_GROUP:
                    gd = min(W_BY_GROUP, max(1, ng // 2))  # DVE group count
                    splits = ((nc.gpsimd, 0, ng - gd), (nc.vector, ng - gd, ng))
                    for eng, g0s, g1s in splits:
                        if g0s == g1s:
                            continue
                        eng.tensor_tensor(
                            out=w4[:, g0s:g1s],
                            in0=ent4[:, g0s:g1s],
                            in1=sc4[:, g0s:g1s].to_broadcast(
                                [128, g1s - g0s, NN, D]
                            ),
                            op=mybir.AluOpType.mult,
                        )
                else:
                    for eng, n0, n1 in (
                        (nc.gpsimd, 0, N_GP),
                        (nc.vector, N_GP, NN),
                    ):
                        if n0 == n1:
                            continue
                        eng.tensor_tensor(
                            out=w4[:, :, n0:n1],
                            in0=ent4[:, :, n0:n1],
                            in1=sc4[:, :, n0:n1].to_broadcast([128, ng, n1 - n0, D]),
                            op=mybir.AluOpType.mult,
                        )

                psum_t = psum_pool.tile([128, ng * D], F32)
                w_nmaj = w_t[:].rearrange("p (gg n d) -> p n gg d", gg=ng, n=NN)
                for c in range(NN):
                    nc.tensor.matmul(
                        out=psum_t[:],
                        lhsT=ident[:],
                        rhs=w_nmaj[:, c : c + 1],
                        start=(c == 0),
                        stop=(c == NN - 1),
                    )
                do = g0 * D
                nc.scalar.copy(out=out_t[:, do : do + ng * D], in_=psum_t[:])

            def compute_sblk(sb):
                out_t = out_pool.tile([128, GG * D], F32)
                if sb == n_sblk - 1:
                    # tapered tail: 4 chunks of 2 groups, output in 2 halves
                    for h in range(4):
                        compute_chunk(sb, out_t, h * 2, 2)
                        if h == 1:
                            nc.sync.dma_start(
                                out=out_v[sb][:, : GG * D // 2],
                                in_=out_t[:, : GG * D // 2],
                            )
                    nc.sync.dma_start(
                        out=out_v[sb][:, GG * D // 2 :],
                        in_=out_t[:, GG * D // 2 :],
                    )
                else:
                    compute_chunk(sb, out_t, 0, 4)
                    compute_chunk(sb, out_t, 4, 4)
                    nc.sync.dma_start(out=out_v[sb], in_=out_t[:])

            for i in range(n_sblk + LOOKAHEAD):
                if i < n_sblk:
                    issue_loads(i)
                if i == 0:
                    nc.gpsimd.dma_start(out=drug_all[:], in_=drug_av)
                if 1 <= i < n_sblk + 1:
                    broadcast_drug(i - 1)
                if i >= LOOKAHEAD:
                    compute_sblk(i - LOOKAHEAD)

    nc.compile()
    return nc


_NC_CACHE: dict = {}


def _cfg_key():
    return (LOOKAHEAD, N_GP, str(IDENT_DT), SCAN0, DB_BUFS, W_BY_GROUP)


def _get_nc(b_local: int = B_LOCAL):
    key = (b_local, _cfg_key())
    if key not in _NC_CACHE:
        _NC_CACHE[key] = _build_nc(b_local)
    return _NC_CACHE[key]


def run_sharded(drug, rel, ent, trace: bool = False):
    """Shard batch dim across the 8 cores, run, gather. Returns
    (full output [B, G, D], BassKernelResults)."""
    drug = np.ascontiguousarray(np.asarray(drug, dtype=np.float32))
    rel = np.ascontiguousarray(np.asarray(rel, dtype=np.float32))
    ent = np.ascontiguousarray(np.asarray(ent, dtype=np.float32))
    b = drug.shape[0]
    nb = b // N_CORES
    assert nb * N_CORES == b
    nc = _get_nc(nb)
    in_maps = [
        {
            "drug": np.ascontiguousarray(drug[i * nb : (i + 1) * nb]),
            "rel": np.ascontiguousarray(rel[i * nb : (i + 1) * nb]),
            "ent": np.ascontiguousarray(ent[i * nb : (i + 1) * nb]),
        }
        for i in range(N_CORES)
    ]
    last_exc = None
    for attempt in range(3):
        try:
            res = run_bass_kernel_spmd(nc, in_maps, list(range(N_CORES)), trace=trace)
            break
        except Exception as exc:  # transient device-unrecoverable states
            last_exc = exc
            import time

            time.sleep(10 * (attempt + 1))
    else:
        raise last_exc
    out = np.concatenate([res.results[i]["out"] for i in range(N_CORES)], axis=0)
    return out, res


def kernel(drug, rel, ent):
    out, _ = run_sharded(drug, rel, ent, trace=False)
    return out
